# revision 3
# baseline (speedup 1.0000x reference)
"""MoE routing gate kernel for Trainium2 (8 NeuronCores, data-parallel).

Problem (hardcoded): x [4, 4096, 2048] f32, w_gate [64, 2048] f32,
expert_bias [64] f32 (zeros per spec).
  gate_logits = x @ w_gate.T          # [B, S, 64]
  gate_weights = sigmoid(gate_logits)
  topk_vals, topk_idx = top_k(gate_logits + bias, k=8)
  topk_weights = gather(gate_weights, topk_idx); normalize
Returns (topk_weights [4,4096,8] f32, topk_indices [4,4096,8] int32).

Strategy: shard the 16384 tokens across 8 cores (2048 each); replicate
w_gate. Host pre-packs each core's x slice into PE-friendly layout
[t, dp, k, tau] = x[token t*128+tau, d=k*128+dp] so the device kernel
streams contiguous 1 MiB tiles and feeds the tensor engine directly
(lhsT = x block with contraction dim D on partitions, no on-device
transpose). Per 128-token tile: 16 fp32 matmuls accumulate logits
[128 tok, 64 exp] in PSUM; ACT copies to SBUF; DVE max/max_index give
the top-8 values+indices; ACT sigmoid (+row-sum), DVE reciprocal and
scalar-mul normalize. Expert bias is zeros per the problem spec, so
biased logits == logits (a numpy fallback guards the general case).
"""

import numpy as np

_B, _S, _D, _E = 4, 4096, 2048, 64
_K = 8
_NCORES = 8
_TOK = _B * _S              # 16384 tokens
_TC = _TOK // _NCORES       # 2048 tokens per core
_NT = _TC // 128            # 16 token tiles per core
_NKC = _D // 128            # 16 contraction chunks

_prog_cache = {}


def _ensure_path():
    import sys
    for p in ("/opt/trn_rl_repo",):
        if p not in sys.path:
            sys.path.insert(0, p)


def _build_program():
    """Build the per-core Bass/Tile program (SPMD: same program, different data)."""
    _ensure_path()
    import concourse.bass as bass
    import concourse.tile as tile
    from concourse import bacc, mybir

    nc = bacc.Bacc("TRN2", target_bir_lowering=False, debug=False,
                   num_devices=_NCORES)

    f32 = mybir.dt.float32
    u32 = mybir.dt.uint32

    # DRAM I/O (per core)
    xt = nc.dram_tensor("xt", [_NT, 128, _NKC * 128], f32, kind="ExternalInput")
    wt = nc.dram_tensor("wt", [128, _NKC * _E], f32, kind="ExternalInput")
    out_w = nc.dram_tensor("out_w", [_NT, 128, _K], f32, kind="ExternalOutput")
    out_i = nc.dram_tensor("out_i", [_NT, 128, _K], u32, kind="ExternalOutput")

    with tile.TileContext(nc) as tc:
        with (
            tc.tile_pool(name="xpool", bufs=3) as xpool,
            tc.tile_pool(name="wpool", bufs=1) as wpool,
            tc.tile_pool(name="psum", bufs=4, space=bass.MemorySpace.PSUM) as psum,
            tc.tile_pool(name="lpool", bufs=4) as lpool,
            tc.tile_pool(name="opool", bufs=4) as opool,
        ):
            wt_sb = wpool.tile([128, _NKC * _E], f32)
            nc.sync.dma_start(wt_sb[:], wt[:])

            for t in range(_NT):
                xtile = xpool.tile([128, _NKC * 128], f32)
                nc.sync.dma_start(xtile[:], xt[t])

                ps = psum.tile([128, _E], f32)
                for k in range(_NKC):
                    nc.tensor.matmul(
                        ps[:],
                        xtile[:, bass.ts(k, 128)],   # lhsT [dp, tau]
                        wt_sb[:, bass.ts(k, _E)],    # rhs  [dp, e]
                        start=(k == 0),
                        stop=(k == _NKC - 1),
                    )

                logit = lpool.tile([128, _E], f32)
                nc.scalar.copy(logit[:], ps[:])

                vals = opool.tile([128, _K], f32)
                nc.vector.max(vals[:], logit[:])
                idx = opool.tile([128, _K], u32)
                nc.vector.max_index(idx[:], vals[:], logit[:])

                sig = opool.tile([128, _K], f32)
                ssum = opool.tile([128, 1], f32)
                nc.scalar.activation(
                    sig[:], vals[:], mybir.ActivationFunctionType.Sigmoid,
                    accum_out=ssum[:],
                )
                rsum = opool.tile([128, 1], f32)
                nc.vector.reciprocal(rsum[:], ssum[:])
                wout = opool.tile([128, _K], f32)
                nc.vector.tensor_scalar_mul(wout[:], sig[:], rsum[:])

                nc.scalar.dma_start(out_w[t], wout[:])
                nc.scalar.dma_start(out_i[t], idx[:])

    nc.compile()
    return nc


def _get_program():
    if "nc" not in _prog_cache:
        _prog_cache["nc"] = _build_program()
    return _prog_cache["nc"]


def _pack_inputs(x, w_gate):
    """Host-side layout transform. Returns per-core input maps."""
    x2 = np.ascontiguousarray(x, dtype=np.float32).reshape(_TOK, _D)
    # wt[dp, k*64+e] = w_gate[e, k*128+dp]
    wt = np.ascontiguousarray(
        w_gate.T.reshape(_NKC, 128, _E).transpose(1, 0, 2).reshape(128, _NKC * _E),
        dtype=np.float32,
    )
    in_maps = []
    for c in range(_NCORES):
        xc = x2[c * _TC:(c + 1) * _TC]                 # [2048 tok, 2048 d]
        # [t, tau, k, dp] -> [t, dp, k, tau]
        xt = np.ascontiguousarray(
            xc.reshape(_NT, 128, _NKC, 128).transpose(0, 3, 2, 1)
        ).reshape(_NT, 128, _NKC * 128)
        in_maps.append({"xt": xt, "wt": wt})
    return in_maps


def _numpy_reference(x, w_gate, expert_bias):
    """Exact fallback for the (unspecced) nonzero-bias case."""
    x2 = np.asarray(x, dtype=np.float32).reshape(_TOK, _D)
    logits = x2 @ np.asarray(w_gate, dtype=np.float32).T
    gw = 1.0 / (1.0 + np.exp(-logits))
    biased = logits + np.asarray(expert_bias, dtype=np.float32)
    idx = np.argsort(-biased, axis=-1, kind="stable")[:, :_K].astype(np.int32)
    tw = np.take_along_axis(gw, idx, axis=-1)
    tw = tw / tw.sum(axis=-1, keepdims=True)
    return (
        tw.reshape(_B, _S, _K).astype(np.float32),
        idx.reshape(_B, _S, _K).astype(np.int32),
    )


def _run(x, w_gate, expert_bias, trace=False, trace_kwargs=None):
    _ensure_path()
    from concourse.bass_utils import run_bass_kernel_spmd

    nc = _get_program()
    in_maps = _pack_inputs(x, w_gate)
    res = run_bass_kernel_spmd(
        nc, in_maps, list(range(_NCORES)), trace=trace,
        **(trace_kwargs or {}),
    )
    w_parts = [r["out_w"].reshape(_TC, _K) for r in res.results]
    i_parts = [r["out_i"].reshape(_TC, _K) for r in res.results]
    weights = np.concatenate(w_parts, axis=0).reshape(_B, _S, _K)
    indices = (
        np.concatenate(i_parts, axis=0).astype(np.int32).reshape(_B, _S, _K)
    )
    return (weights, indices), res


def kernel(x, w_gate, expert_bias):
    x = np.asarray(x)
    w_gate = np.asarray(w_gate)
    expert_bias = np.asarray(expert_bias)
    assert x.shape == (_B, _S, _D), x.shape
    assert w_gate.shape == (_E, _D), w_gate.shape
    if np.any(expert_bias):
        # Spec pins expert_bias to zeros; keep a correct host path anyway.
        return _numpy_reference(x, w_gate, expert_bias)
    (weights, indices), _ = _run(x, w_gate, expert_bias)
    return weights, indices


# revision 7
# speedup vs baseline: 1.1793x; 1.1793x over previous
"""MoE routing gate kernel for Trainium2 (8 NeuronCores, data-parallel).

Problem (hardcoded): x [4, 4096, 2048] f32, w_gate [64, 2048] f32,
expert_bias [64] f32 (zeros per spec).
  gate_logits = x @ w_gate.T          # [B, S, 64]
  gate_weights = sigmoid(gate_logits)
  topk_vals, topk_idx = top_k(gate_logits + bias, k=8)
  topk_weights = gather(gate_weights, topk_idx); normalize
Returns (topk_weights [4,4096,8] f32, topk_indices [4,4096,8] int32).

Strategy: shard the 16384 tokens across 8 cores (2048 each); replicate
w_gate. Host pre-packs each core's x slice into a PE-friendly layout
[k, dp, g, tau] = x[token g*512+tau, d = k*128+dp], so the device
kernel streams large contiguous tiles from HBM straight into the
tensor engine's *moving* operand (the fast path for fp32) with the
small router weight as the stationary operand:
  psum_g[64 e, 512 tok] += wt_k[128 dp, 64 e].T @ x_k[128 dp, 512 tok]
accumulated over the 16 contraction chunks k into 4 PSUM banks.
Logits are then re-transposed token-major via 128x128 PE transposes,
and per 128-token tile the DVE max/max_index ops give the top-8
values+indices; ACT sigmoid (+row-sum), DVE reciprocal and scalar-mul
normalize. Expert bias is zeros per the problem spec, so biased
logits == logits (a numpy fallback guards the general case).
"""

import numpy as np

_B, _S, _D, _E = 4, 4096, 2048, 64
_K = 8
_NCORES = 8
_TOK = _B * _S              # 16384 tokens
_TC = _TOK // _NCORES       # 2048 tokens per core
_NG = 4                     # token groups of 512 per core
_GT = 512                   # tokens per group (PSUM bank / fp32 moving max)
_NKC = _D // 128            # 16 contraction chunks
_KGROUPS = (1, 1, 2, 4, 4, 4)   # k-chunks per DMA (graduated prefetch)

_prog_cache = {}


def _ensure_path():
    import sys
    for p in ("/opt/trn_rl_repo",):
        if p not in sys.path:
            sys.path.insert(0, p)


def _build_program(mode="f32"):
    """Per-core Bass/Tile program (SPMD: same program, different data)."""
    _ensure_path()
    import concourse.bass as bass
    import concourse.tile as tile
    from concourse import bacc, mybir

    nc = bacc.Bacc("TRN2", target_bir_lowering=False, debug=False,
                   num_devices=_NCORES)

    f32 = mybir.dt.float32
    u32 = mybir.dt.uint32
    mm_dt = mybir.dt.float32r if mode == "f32r" else f32

    # DRAM I/O (per core)
    xa = nc.dram_tensor("xa", [_NKC, 128, _NG * _GT], f32, kind="ExternalInput")
    wt = nc.dram_tensor("wt", [128, _NKC * _E], f32, kind="ExternalInput")
    ident = nc.dram_tensor("ident", [_E, _E], f32, kind="ExternalInput")
    out_w = nc.dram_tensor("out_w", [128, _NG * _NG, _K], f32,
                           kind="ExternalOutput")
    out_i = nc.dram_tensor("out_i", [128, _NG * _NG, _K], u32,
                           kind="ExternalOutput")

    with tile.TileContext(nc) as tc:
        with (
            tc.tile_pool(name="xpool", bufs=2) as xpool,
            tc.tile_pool(name="wpool", bufs=1) as wpool,
            tc.tile_pool(name="psA", bufs=1, space=bass.MemorySpace.PSUM) as psA,
            tc.tile_pool(name="psB", bufs=2, space=bass.MemorySpace.PSUM) as psB,
            tc.tile_pool(name="lpool", bufs=2) as lpool,
            tc.tile_pool(name="opool", bufs=1) as opool,
            tc.tile_pool(name="tpool", bufs=4) as tpool,
        ):
            # One-time loads on the ACT HWDGE ring so they don't head-block
            # the first x chunk on the sync ring.
            wt_sb = wpool.tile([128, _NKC * _E], f32)
            nc.scalar.dma_start(wt_sb[:], wt[:])
            id_sb = wpool.tile([_E, _E], f32)
            nc.scalar.dma_start(id_sb[:], ident[:])

            # Accumulators: 4 PSUM banks, one per 512-token group.
            ps_all = psA.tile([_E, _NG, _GT], f32)
            # Output accumulation tiles (single store at the end).
            wacc = opool.tile([128, _NG * _NG, _K], f32)
            iacc = opool.tile([128, _NG * _NG, _K], u32)

            # Matmul phase: k-outer (weights stationary per chunk),
            # groups inner, PSUM accumulation over k.
            k0 = 0
            for ng in _KGROUPS:
                xt = xpool.tile([128, 4 * 2048], f32, tag="xa")
                nc.sync.dma_start(
                    xt[:, :ng * 2048].rearrange("p (k n) -> p k n", k=ng),
                    xa[k0:k0 + ng].rearrange("k p n -> p k n"),
                )
                for ks in range(ng):
                    k = k0 + ks
                    for g in range(_NG):
                        lhsT = wt_sb[:, bass.ts(k, _E)]
                        rhs = xt[:, ks * 2048 + g * _GT:
                                 ks * 2048 + (g + 1) * _GT]
                        if mode == "f32r":
                            lhsT = lhsT.bitcast(mm_dt)
                            rhs = rhs.bitcast(mm_dt)
                        nc.tensor.matmul(
                            ps_all[:, g, :], lhsT, rhs,
                            start=(k == 0), stop=(k == _NKC - 1),
                        )
                k0 += ng

            # Post-processing per group: copy logitsT to SBUF, PE-transpose
            # back to token-major, then top-8 + sigmoid + normalize.
            for g in range(_NG):
                lg = lpool.tile([_E, _GT], f32, tag="lg")
                nc.scalar.copy(lg[:], ps_all[:, g, :])

                ps2 = psB.tile([128, _NG, _E], f32, tag="ps2")
                for j in range(_NG):
                    nc.tensor.transpose(
                        ps2[:, j, :], lg[:, bass.ts(j, 128)], id_sb[:],
                    )
                lg2 = lpool.tile([128, _NG, _E], f32, tag="lg2")
                nc.scalar.copy(lg2[:], ps2[:])

                for j in range(_NG):
                    t = g * _NG + j
                    logit = lg2[:, j, :]
                    vals = tpool.tile([128, _K], f32, tag="vals")
                    nc.vector.max(vals[:], logit)
                    nc.vector.max_index(iacc[:, t, :], vals[:], logit)

                    sig = tpool.tile([128, _K], f32, tag="sig")
                    ssum = tpool.tile([128, 1], f32, tag="ssum")
                    nc.scalar.activation(
                        sig[:], vals[:], mybir.ActivationFunctionType.Sigmoid,
                        accum_out=ssum[:],
                    )
                    rsum = tpool.tile([128, 1], f32, tag="rsum")
                    nc.vector.reciprocal(rsum[:], ssum[:])
                    nc.vector.tensor_scalar_mul(wacc[:, t, :], sig[:], rsum[:])

            nc.scalar.dma_start(out_w[:], wacc[:])
            nc.scalar.dma_start(out_i[:], iacc[:])

    nc.compile()
    return nc


def _get_program(mode="f32"):
    if mode not in _prog_cache:
        _prog_cache[mode] = _build_program(mode)
    return _prog_cache[mode]


def _pack_inputs(x, w_gate):
    """Host-side layout transform. Returns per-core input maps."""
    x2 = np.ascontiguousarray(x, dtype=np.float32).reshape(_TOK, _D)
    # wt[dp, k*64+e] = w_gate[e, k*128+dp]
    wt = np.ascontiguousarray(
        w_gate.T.reshape(_NKC, 128, _E).transpose(1, 0, 2).reshape(128, _NKC * _E),
        dtype=np.float32,
    )
    ident = np.eye(_E, dtype=np.float32)
    in_maps = []
    for c in range(_NCORES):
        xc = x2[c * _TC:(c + 1) * _TC]                 # [2048 tok, 2048 d]
        # [g, tau, k, dp] -> [k, dp, g, tau]
        xa = np.ascontiguousarray(
            xc.reshape(_NG, _GT, _NKC, 128).transpose(2, 3, 0, 1)
        ).reshape(_NKC, 128, _NG * _GT)
        in_maps.append({"xa": xa, "wt": wt, "ident": ident})
    return in_maps


def _unpack_outputs(results):
    w_parts, i_parts = [], []
    for r in results:
        # [128 tau, 16 t, 8] -> token t*128+tau -> [2048, 8]
        w_parts.append(r["out_w"].transpose(1, 0, 2).reshape(_TC, _K))
        i_parts.append(r["out_i"].transpose(1, 0, 2).reshape(_TC, _K))
    weights = np.concatenate(w_parts, axis=0).reshape(_B, _S, _K)
    indices = (
        np.concatenate(i_parts, axis=0).astype(np.int32).reshape(_B, _S, _K)
    )
    return weights, indices


def _numpy_reference(x, w_gate, expert_bias):
    """Exact fallback for the (unspecced) nonzero-bias case."""
    x2 = np.asarray(x, dtype=np.float32).reshape(_TOK, _D)
    logits = x2 @ np.asarray(w_gate, dtype=np.float32).T
    gw = 1.0 / (1.0 + np.exp(-logits))
    biased = logits + np.asarray(expert_bias, dtype=np.float32)
    idx = np.argsort(-biased, axis=-1, kind="stable")[:, :_K].astype(np.int32)
    tw = np.take_along_axis(gw, idx, axis=-1)
    tw = tw / tw.sum(axis=-1, keepdims=True)
    return (
        tw.reshape(_B, _S, _K).astype(np.float32),
        idx.reshape(_B, _S, _K).astype(np.int32),
    )


def _run(x, w_gate, expert_bias, trace=False, mode="f32", trace_kwargs=None):
    _ensure_path()
    from concourse.bass_utils import run_bass_kernel_spmd

    nc = _get_program(mode)
    in_maps = _pack_inputs(x, w_gate)
    res = run_bass_kernel_spmd(
        nc, in_maps, list(range(_NCORES)), trace=trace,
        **(trace_kwargs or {}),
    )
    weights, indices = _unpack_outputs(res.results)
    return (weights, indices), res


def kernel(x, w_gate, expert_bias):
    x = np.asarray(x)
    w_gate = np.asarray(w_gate)
    expert_bias = np.asarray(expert_bias)
    assert x.shape == (_B, _S, _D), x.shape
    assert w_gate.shape == (_E, _D), w_gate.shape
    if np.any(expert_bias):
        # Spec pins expert_bias to zeros; keep a correct host path anyway.
        return _numpy_reference(x, w_gate, expert_bias)
    (weights, indices), _ = _run(x, w_gate, expert_bias)
    return weights, indices
